# revision 8
# baseline (speedup 1.0000x reference)
"""Trainium2 Bass kernel for nn_DynamicConv2d: per-sample dynamic conv.

  feat = x.mean(H,W); h1 = relu(feat@w1+b1); wgen = (h1@w2+b2) -> per-sample
  [COUT, CIN, 3, 3] conv weights; out[s] = conv2d(x[s], wgen[s], pad=1).

Sharding: batch B=32 across 8 cores (4 samples/core), MLP params replicated.

v1 "X-stationary" scheme (cost model charges matmuls by out-free-size only,
so the win comes from putting 128 useful contraction lanes AND 128 useful
output partitions to work per streamed column):

  - per sample a dup tile [128p, 130, 130] bf16: parts 0-63 = x zero-padded
    (host-padded), parts 64-127 = the same image shifted one column left
    (DVE copy).  A [128, 128] row window of this tile is the matmul
    STATIONARY: lower half covers offset (dy,0), upper half (dy,1).
  - conv: per output row h, 6 matmuls accumulate psum[w=128, co=64]:
      3 "pair" passes  (dy,0)+(dy,1), contraction 128, lhsT=dup[:,h+dy,0:128]
      3 "single" passes (dy,2),       contraction 64,  lhsT=dup[0:64,h+dy,2:130]
    moving = per-sample weight tiles wp[s][dy] [128,64] / ws[s][dy] [64,64].
    Total conv PE rows: 4*128*6*64 = 196k vs 295k for the block-diag scheme.
  - out psum is [w, co]; staged bf16 and DMA'd to HBM [s, w, h, co] (h,co
    contiguous => 2KB descriptors, no small-elem penalty); host transposes
    back to [s, co, h, w] for free.
  - wgen: same 18-slice pipeline as before (4 tile-position-packed matmuls +
    DVE StreamTranspose per slice), but assembled into per-sample per-dy
    moving tiles; b2 added per completed tile.
  - feat: chunked reduces chasing the x DMA, spread over ACT/DVE/Pool so the
    last sample's feat closes ~2us after its DMA.
"""

import sys

for _p in ("/opt/trn_rl_repo",):
    if _p not in sys.path:
        sys.path.insert(0, _p)

from contextlib import ExitStack

import numpy as np

import concourse.bass as bass
import concourse.tile as tile
from concourse import bacc, mybir
from concourse.bass_utils import run_bass_kernel_spmd

F32 = mybir.dt.float32
BF16 = mybir.dt.bfloat16

B, CIN, COUT, K, H, W = 32, 64, 64, 3, 128, 128
NCORES = 8
BSH = B // NCORES          # 4 samples per core
HID = 128                  # MLP hidden
JTOT = COUT * CIN * K * K  # 36864
NOFF = K * K               # 9
HP = H + 2                 # height-padded image rows
WP2 = W + 2                # width-padded image cols
HW = H * W

# w2 slice order: pair offsets (dx 0/1) first, singles (dx=2) last, so the
# pair weight tiles complete early (enables conv pre-run in later versions).
OFF_ORDER = [(0, 0), (0, 1), (1, 0), (1, 1), (2, 0), (2, 1), (0, 2), (1, 2), (2, 2)]

NXC = 5                    # x sub-DMAs per sample (130 rows = 5 x 26)
XROWS = HP // NXC


def build_kernel_body(nc, tc, ctx, aps):
    x_ap = aps["x"]      # [BSH, CIN, HP, WP2] bf16 (host zero-padded)
    w1_ap = aps["w1"]    # [CIN, HID] f32
    b1_ap = aps["b1"]    # [HID, 1] f32
    w2_ap = aps["w2"]    # [HID, NOFF, COUT, CIN] bf16 (host OFF_ORDER-major)
    b2p_ap = aps["b2p"]  # [2*CIN, 3, COUT] bf16: [64*dx+ci, dy, co]
    b2s_ap = aps["b2s"]  # [CIN, 3, COUT] bf16: [ci, dy, co] for dx=2
    out_ap = aps["out"]  # [BSH, W, H, COUT] bf16 (w-major; host untransposes)

    const = ctx.enter_context(tc.tile_pool(name="const", bufs=1))
    dpool = ctx.enter_context(tc.tile_pool(name="dpool", bufs=1))
    w2pool = ctx.enter_context(tc.tile_pool(name="w2pool", bufs=4))
    tpool = ctx.enter_context(tc.tile_pool(name="tpool", bufs=2))
    fpool = ctx.enter_context(tc.tile_pool(name="fpool", bufs=3))
    outp = ctx.enter_context(tc.tile_pool(name="outp", bufs=3))
    mlp_ps = ctx.enter_context(tc.tile_pool(name="mlp_ps", bufs=1, space="PSUM"))
    wg_ps = ctx.enter_context(tc.tile_pool(name="wg_ps", bufs=2, space="PSUM"))
    cv_ps = ctx.enter_context(tc.tile_pool(name="cv_ps", bufs=4, space="PSUM"))

    # ---- x DMA chain first (SP queue owns HBM bandwidth from t=0) ----
    dup = []
    for s in range(BSH):
        dup.append(dpool.tile([2 * CIN, HP, WP2], BF16, name=f"dup{s}"))
    for s in range(BSH):
        for c in range(NXC):
            r0, r1 = c * XROWS, (c + 1) * XROWS
            nc.sync.dma_start(
                out=dup[s][0:CIN, r0:r1, :], in_=x_ap[s, :, r0:r1, :]
            )

    # ---- tiny zero-init + small params (ACT HWDGE queue) ----
    h1T32 = const.tile([HID, 32], BF16)
    nc.vector.memset(h1T32, 0.0)
    w1_sb = const.tile([CIN, HID], F32)
    nc.scalar.dma_start(out=w1_sb, in_=w1_ap)
    b1_sb = const.tile([HID, 1], F32)
    nc.scalar.dma_start(out=b1_sb, in_=b1_ap)
    b2p_sb = const.tile([2 * CIN, 3, COUT], BF16)
    nc.scalar.dma_start(out=b2p_sb, in_=b2p_ap)
    b2s_sb = const.tile([CIN, 3, COUT], BF16)
    nc.scalar.dma_start(out=b2s_sb, in_=b2s_ap)

    # ---- dup copies + feat, chasing the x DMAs.
    # s0/s1: dup right away (DVE), feat via ACT accumulate.
    # s2/s3: feat via DVE bf16 fold-tree (fast close -> h1 by ~31us);
    #        their dup copies are deferred into the wgen window.
    fsum = const.tile([CIN, BSH], F32)
    HR = XROWS // 2  # 13

    for s in (0, 1):
        t = dup[s]
        fpart = fpool.tile([CIN, NXC], F32, tag="fpart", name=f"fpart{s}")
        for c in range(NXC):
            r0, r1 = c * XROWS, (c + 1) * XROWS
            nc.vector.tensor_copy(
                out=t[CIN : 2 * CIN, r0:r1, 0 : WP2 - 1], in_=t[0:CIN, r0:r1, 1:WP2]
            )
            ascr = fpool.tile(
                [CIN, XROWS * WP2], BF16, tag="ascr", name=f"ascr{s}_{c}"
            )
            nc.scalar.activation(
                out=ascr,
                in_=t[0:CIN, r0:r1, :],
                func=mybir.ActivationFunctionType.Copy,
                accum_out=fpart[:, c : c + 1],
            )
        nc.vector.tensor_reduce(
            out=fsum[:, s : s + 1],
            in_=fpart,
            axis=mybir.AxisListType.X,
            op=mybir.AluOpType.add,
        )

    tree = {}
    for s in (2, 3):
        t = dup[s]
        part = const.tile([CIN, HR, WP2], BF16, name=f"tpart{s}")
        tree[s] = part
        for c in range(NXC):
            r0 = c * XROWS
            if c == 0:
                nc.vector.tensor_tensor(
                    out=part,
                    in0=t[0:CIN, r0 : r0 + HR, :],
                    in1=t[0:CIN, r0 + HR : r0 + XROWS, :],
                    op=mybir.AluOpType.add,
                )
            else:
                fscr = fpool.tile([CIN, HR, WP2], BF16, tag="fscr", name=f"fscr{s}_{c}")
                nc.vector.tensor_tensor(
                    out=fscr,
                    in0=t[0:CIN, r0 : r0 + HR, :],
                    in1=t[0:CIN, r0 + HR : r0 + XROWS, :],
                    op=mybir.AluOpType.add,
                )
                nc.vector.tensor_tensor(
                    out=part, in0=part, in1=fscr, op=mybir.AluOpType.add
                )
        nc.vector.tensor_reduce(
            out=fsum[:, s : s + 1],
            in_=part,
            axis=mybir.AxisListType.XY,
            op=mybir.AluOpType.add,
        )

    # ---- MLP: h1 = relu((feat_sums/HW) @ w1 + b1) for all 4 samples ----
    w1s = const.tile([CIN, HID], F32)
    nc.scalar.mul(out=w1s, in_=w1_sb, mul=1.0 / HW)
    h1_ps = mlp_ps.tile([HID, BSH], F32)
    nc.tensor.matmul(out=h1_ps, lhsT=w1s, rhs=fsum, start=True, stop=True)
    nc.scalar.activation(
        out=h1T32[:, 0:BSH],
        in_=h1_ps,
        func=mybir.ActivationFunctionType.Relu,
        bias=b1_sb,
        scale=1.0,
    )

    # ---- wgen: 18 (offset, co-half) slices -> per-sample moving tiles ----
    wp = [[const.tile([2 * CIN, COUT], BF16, name=f"wp{s}_{dy}") for dy in range(3)]
          for s in range(BSH)]
    ws = [[const.tile([CIN, COUT], BF16, name=f"ws{s}_{dy}") for dy in range(3)]
          for s in range(BSH)]

    # deferred dup copies for s2/s3, drip-fed into the DVE stream between
    # transposes (they have no pending deps, so they fill DVE idle slots)
    dup_pending = [
        (s, c * XROWS, (c + 1) * XROWS) for s in (2, 3) for c in range(NXC)
    ]

    def emit_dup_chunk():
        if dup_pending:
            s, r0, r1 = dup_pending.pop(0)
            nc.vector.tensor_copy(
                out=dup[s][CIN : 2 * CIN, r0:r1, 0 : WP2 - 1],
                in_=dup[s][0:CIN, r0:r1, 1:WP2],
            )

    for k in range(NOFF):
        dy, dx = OFF_ORDER[k]
        for half in range(2):
            w2sl = w2pool.tile(
                [HID, 32, CIN], BF16, tag="w2sl", name=f"w2sl{k}_{half}"
            )
            nc.sync.dma_start(
                out=w2sl, in_=w2_ap[:, k, 32 * half : 32 * (half + 1), :]
            )
            wps = wg_ps.tile([2 * CIN, 512], F32, tag="wps", name=f"wps{k}_{half}")
            for g in range(4):  # (co-16 q, ci-half)
                q, cih = g // 2, g % 2
                nc.tensor.matmul(
                    out=wps[32 * g : 32 * (g + 1), :],
                    lhsT=h1T32,
                    rhs=w2sl[:, 16 * q : 16 * (q + 1), 32 * cih : 32 * (cih + 1)],
                    start=True,
                    stop=True,
                    tile_position=(0, 32 * g),
                )
            tmid = tpool.tile([2 * CIN, 512], F32, tag="tmid", name=f"tmid{k}_{half}")
            nc.vector.transpose(out=tmid, in_=wps)
            if half == 1:
                emit_dup_chunk()
            tr = tmid.rearrange("p (co s) -> p co s", co=16, s=32)
            for s in range(BSH):
                for q in range(2):
                    co0 = 32 * half + 16 * q
                    src = tr[64 * q : 64 * (q + 1), :, s : s + 1]
                    if dx < 2:
                        dst = wp[s][dy][64 * dx : 64 * dx + CIN, co0 : co0 + 16]
                    else:
                        dst = ws[s][dy][:, co0 : co0 + 16]
                    nc.gpsimd.tensor_copy(out=dst, in_=src)
        # a (s, dy) moving tile is complete once both halves of its last
        # offset landed; add b2 in place (same-base-partition TensorTensor)
        if dx == 1:
            for s in range(BSH):
                nc.vector.tensor_tensor(
                    out=wp[s][dy],
                    in0=wp[s][dy],
                    in1=b2p_sb[:, dy, :],
                    op=mybir.AluOpType.add,
                )
        elif dx == 2:
            for s in range(BSH):
                nc.vector.tensor_tensor(
                    out=ws[s][dy],
                    in0=ws[s][dy],
                    in1=b2s_sb[:, dy, :],
                    op=mybir.AluOpType.add,
                )

    while dup_pending:
        emit_dup_chunk()

    # ---- conv: per (sample, output row) 6 matmuls into psum[w, co] ----
    out_v = out_ap.rearrange("s w h c -> w s (h c)")  # [W, BSH, H*COUT]
    GR = 8                     # rows per psum group (one 2KB bank)

    def conv_group(s, g):
        t = dup[s]
        pt = cv_ps.tile([W, GR, COUT], F32, tag="pt", name=f"pt{s}_{g}")
        for r in range(GR):
            h = GR * g + r
            po = pt[:, r, :]
            for dy in range(3):
                nc.tensor.matmul(
                    out=po,
                    lhsT=t[:, h + dy, 0:W],
                    rhs=wp[s][dy],
                    start=(dy == 0),
                    stop=False,
                )
            for dy in range(3):
                nc.tensor.matmul(
                    out=po,
                    lhsT=t[0:CIN, h + dy, 2 : 2 + W],
                    rhs=ws[s][dy],
                    start=False,
                    stop=(dy == 2),
                )
        return pt

    def emit_super(s, g0, ng):
        # ng psum groups -> one staging tile -> one out DMA
        ost = outp.tile([W, ng * GR * COUT], BF16, tag=f"ost{ng}", name=f"ost{s}_{g0}")
        for i in range(ng):
            pt = conv_group(s, g0 + i)
            nc.scalar.copy(out=ost[:, i * GR * COUT : (i + 1) * GR * COUT], in_=pt)
        h0 = GR * g0
        nc.sync.dma_start(
            out=out_v[:, s, h0 * COUT : (h0 + ng * GR) * COUT], in_=ost
        )

    NG = H // GR  # 16 psum groups per sample
    for s in range(BSH):
        if s < BSH - 1:
            for gg in range(NG // 2):
                emit_super(s, 2 * gg, 2)
        else:
            # final sample: shrink the tail so the last drain+DMA is short
            for gg in range(NG // 2 - 1):
                emit_super(s, 2 * gg, 2)
            emit_super(s, NG - 2, 1)
            emit_super(s, NG - 1, 1)


_CACHE = {}


def build_nc():
    if "nc" in _CACHE:
        return _CACHE["nc"], _CACHE["aps"]
    nc = bacc.Bacc("TRN2", debug=False, num_devices=NCORES)
    aps = {
        "x": nc.dram_tensor("x", [BSH, CIN, HP, WP2], BF16, kind="ExternalInput").ap(),
        "w1": nc.dram_tensor("w1", [CIN, HID], F32, kind="ExternalInput").ap(),
        "b1": nc.dram_tensor("b1", [HID, 1], F32, kind="ExternalInput").ap(),
        "w2": nc.dram_tensor(
            "w2", [HID, NOFF, COUT, CIN], BF16, kind="ExternalInput"
        ).ap(),
        "b2p": nc.dram_tensor("b2p", [2 * CIN, 3, COUT], BF16, kind="ExternalInput").ap(),
        "b2s": nc.dram_tensor("b2s", [CIN, 3, COUT], BF16, kind="ExternalInput").ap(),
        "out": nc.dram_tensor(
            "out", [BSH, W, H, COUT], BF16, kind="ExternalOutput"
        ).ap(),
    }
    with tile.TileContext(nc) as tc, ExitStack() as ctx:
        build_kernel_body(nc, tc, ctx, aps)
    nc.compile()
    _CACHE["nc"] = nc
    _CACHE["aps"] = aps
    return nc, aps


def make_in_maps(x, w1, b1, w2, b2):
    import ml_dtypes

    x = np.asarray(x, dtype=np.float32)
    xpad = np.zeros((B, CIN, HP, WP2), dtype=ml_dtypes.bfloat16)
    xpad[:, :, 1 : H + 1, 1 : W + 1] = x.astype(ml_dtypes.bfloat16)
    w1 = np.ascontiguousarray(np.asarray(w1, dtype=np.float32))
    b1 = np.ascontiguousarray(np.asarray(b1, dtype=np.float32)).reshape(HID, 1)

    # w2 -> [HID, k(OFF_ORDER), co, ci]
    w2r = np.asarray(w2, dtype=np.float32).reshape(HID, COUT, CIN, K, K)
    w2o = w2r.transpose(0, 3, 4, 1, 2).reshape(HID, NOFF, COUT, CIN)
    ko = [3 * dy + dx for (dy, dx) in OFF_ORDER]
    w2o = np.ascontiguousarray(w2o[:, ko].astype(ml_dtypes.bfloat16))

    # b2 -> pair tile [64*dx+ci, dy, co] and single tile [ci, dy, co]
    b2v = np.asarray(b2, dtype=np.float32).reshape(COUT, CIN, K, K)
    b2p = np.zeros((2 * CIN, 3, COUT), dtype=np.float32)
    for dx in range(2):
        b2p[64 * dx : 64 * dx + CIN] = b2v[:, :, :, dx].transpose(1, 2, 0)
    b2s = np.ascontiguousarray(
        b2v[:, :, :, 2].transpose(1, 2, 0).astype(ml_dtypes.bfloat16)
    )
    b2p = np.ascontiguousarray(b2p.astype(ml_dtypes.bfloat16))

    in_maps = []
    for c in range(NCORES):
        in_maps.append(
            {
                "x": np.ascontiguousarray(xpad[c * BSH : (c + 1) * BSH]),
                "w1": w1,
                "b1": b1,
                "w2": w2o,
                "b2p": b2p,
                "b2s": b2s,
            }
        )
    return in_maps


def kernel(x, w1, b1, w2, b2, _trace=False, _results_out=None):
    nc, _ = build_nc()
    in_maps = make_in_maps(x, w1, b1, w2, b2)
    res = run_bass_kernel_spmd(
        nc, in_maps, core_ids=list(range(NCORES)), trace=_trace
    )
    if _results_out is not None:
        _results_out.append(res)
    # out arrives [BSH, W, H, CO] bf16 per core -> [B, CO, H, W] f32
    out = np.concatenate([np.asarray(r["out"]) for r in res.results], axis=0)
    return out.transpose(0, 3, 2, 1).astype(np.float32)


if __name__ == "__main__":
    rng = np.random.default_rng(0)
    ins = {
        "x": rng.standard_normal((B, CIN, H, W)).astype(np.float32),
        "w1": (rng.standard_normal((CIN, HID)) * 0.05).astype(np.float32),
        "b1": (rng.standard_normal((HID,)) * 0.05).astype(np.float32),
        "w2": (rng.standard_normal((HID, JTOT)) * 0.05).astype(np.float32),
        "b2": (rng.standard_normal((JTOT,)) * 0.05).astype(np.float32),
    }
    out = kernel(**ins)
    print("out", out.shape, out.dtype, np.abs(out).mean())
